# revision 1
# baseline (speedup 1.0000x reference)
"""Trainium2 Bass kernel for the distance-transform loss.

Computes, for inputs/targets of shape (16, 1, 512, 512):
    wmse = ALPHA * mean(weight * (inputs - targets)^2)
    dice = BETA  * (1 - (2*I + S) / (U + S))
where weight is built from the per-sample Euclidean distance transform
(EDT) of targets (distance to nearest zero pixel):
    v_b   = max(edt_b),  row_b = edt_b[row=b, :],  mask = (edt != 0)
    weight = mask * (v_b - row_b[w]) + EPS_W

Key reductions (validated against the reference in fp64):
  * mask == targets exactly (edt==0 iff target pixel == 0).
  * sum(weight * e) = sum_b [ v_b * S2_b - dot(sqrt(Drow_b), C_b) ]
    + EPS_W * S1, with e=(x-t)^2, C_b[w] = sum_h (t*e)[h,w],
    S2_b = sum_w C_b, S1 = sum e.
  * Only max(edt^2) and one row per image are needed, so edt^2 is
    computed as: vertical distance clamped at 6 (recursive doubling
    along the free dim in a W-on-partitions layout), then a squared-
    parabolic min over a +/-5 column window (free-dim in the standard
    layout). Exact unless an image contains an all-ones disk of radius
    5 (P ~ 1e-18 for iid uniform 0/1 targets). All distance values are
    small integers, exact in bf16.

Sharding: data-parallel, 2 images per core on 8 cores; per-core scalar
partials are combined on the host (the all-reduce-mean step).
"""

import os
from contextlib import ExitStack

import ml_dtypes
import numpy as np

import concourse.bacc as bacc
import concourse.bass as bass
import concourse.bass_isa as bass_isa
import concourse.mybir as mybir
import concourse.tile as tile
from concourse.bass_utils import run_bass_kernel_spmd

# Problem constants (hardcoded per the task contract).
B, C, H, W = 16, 1, 512, 512
NCORES = 8
IMGS = B // NCORES          # images per core
CB = 4                      # 512 rows = 4 blocks of 128: h = 128*c + p
P = 128
R = 5                       # pass-2 column window radius
DCLAMP = 6.0                # vertical distance clamp (> R)
BIGD = 512.0                # "infinity" for the distance init (bf16-exact)
EPS_W = 1e-3
SMOOTH = 1e-6
ALPHA = 0.6
BETA = 1.0

F32 = mybir.dt.float32
BF16 = mybir.dt.bfloat16
AOP = mybir.AluOpType
ACT = mybir.ActivationFunctionType
AXL = mybir.AxisListType

# Output scalar layout per core: [maxD_0, maxD_1, dot_0, dot_1, S2_0, S2_1,
#                                 S1, sumP, sumPT, sumT, pad...]
OUT_N = 16


def _band_weights():
    """Banded 8^-|k| weights (|k| <= 4) for the PE vertical pass:
    [:, 0, :] within-block, [:, 1, :] from block above, [:, 2, :] below."""
    bm = np.zeros((P, 3, P), np.float32)
    for p in range(P):
        lo, hi = max(0, p - 4), min(P - 1, p + 4)
        for m in range(lo, hi + 1):
            bm[p, 0, m] = 8.0 ** (-abs(p - m))
    for m in range(0, 4):
        for p in range(124 + m, P):
            bm[p, 1, m] = 8.0 ** (-(m + 128 - p))
    for m in range(124, P):
        for p in range(0, m - 124 + 1):
            bm[p, 2, m] = 8.0 ** (-(128 - m + p))
    return bm.astype(ml_dtypes.bfloat16)


def _build_nc():
    nc = bacc.Bacc(
        "TRN2",
        target_bir_lowering=False,
        debug=False,
        num_devices=NCORES,
    )

    x_dram = nc.dram_tensor("x", [IMGS, H, W], F32, kind="ExternalInput")
    t_dram = nc.dram_tensor("t", [IMGS, H, W], F32, kind="ExternalInput")
    sel_dram = nc.dram_tensor("sel", [P, IMGS], F32, kind="ExternalInput")
    res_dram = nc.dram_tensor("res", [1, OUT_N], F32, kind="ExternalOutput")

    dbg_edt = os.environ.get("KERNEL_DEBUG_EDT") == "1"
    if dbg_edt:
        edt_dram = nc.dram_tensor("edt2", [IMGS, H, W], F32, kind="ExternalOutput")

    with tile.TileContext(nc) as tc, ExitStack() as ctx:
        io = ctx.enter_context(tc.tile_pool(name="io", bufs=1))
        bpool = ctx.enter_context(tc.tile_pool(name="b16", bufs=1))
        dpool = ctx.enter_context(tc.tile_pool(name="dping", bufs=2))
        stage = ctx.enter_context(tc.tile_pool(name="stage", bufs=2))
        small = ctx.enter_context(tc.tile_pool(name="small", bufs=1))
        psum = ctx.enter_context(
            tc.tile_pool(name="psum", bufs=1, space=bass.MemorySpace.PSUM)
        )

        SH4 = [P, IMGS, CB, W]   # standard layout: (p, i, c, w), h = 128c+p

        # ---- loads (standard layout; 2KB contiguous rows) ----
        # t first: the whole distance pipeline hangs off it. x only feeds
        # the loss maps, which fill engine gaps later.
        xf = io.tile(SH4, F32, tag="xf")
        tf = io.tile(SH4, F32, tag="tf")
        x_src = x_dram.ap().rearrange("i (c p) w -> p i c w", p=P)
        t_src = t_dram.ap().rearrange("i (c p) w -> p i c w", p=P)
        self32 = small.tile([P, IMGS], F32, tag="self32")
        nc.sync.dma_start(self32[:], sel_dram.ap())
        for i in range(IMGS):
            nc.sync.dma_start(tf[:, i, :, :], t_src[:, i, :, :])
        for i in range(IMGS):
            nc.scalar.dma_start(xf[:, i, :, :], x_src[:, i, :, :])

        selb = small.tile([P, IMGS], BF16, tag="selb")
        nc.scalar.copy(selb[:], self32[:])
        ones_b = small.tile([P, 1], BF16, tag="onesb")
        nc.gpsimd.memset(ones_b[:], 1.0)
        ones_f = small.tile([P, 1], F32, tag="onesf")
        nc.gpsimd.memset(ones_f[:], 1.0)
        fives_f = small.tile([P, 1], F32, tag="fivesf")
        nc.gpsimd.memset(fives_f[:], 5.0)

        # bf16 conversions (per-image for t so each image's transposes can
        # start as soon as its conversion lands)
        xb = bpool.tile(SH4, BF16, tag="xb")
        tb = bpool.tile(SH4, BF16, tag="tb")
        nc.scalar.copy(xb[:], xf[:])

        # Banded weight matrices for the vertical pass: S = sum_k 8^-|k| z
        # within +/-4 rows, contracted over partitions by the PE. Exact
        # powers of two, so bf16/f32 arithmetic is exact enough that
        # thresholds at 0.5*8^-r recover min(vertical distance, 5).
        bmat_dram = nc.inline_tensor(_band_weights(), name="bweights")
        bsb = small.tile([P, 3, P], BF16, tag="bsb")
        nc.sync.dma_start(bsb[:], bmat_dram.ap())

        # ---- per-image distance pipeline (pipelined across images) ----
        Ds = []
        for i in range(IMGS):
            nc.scalar.copy(tb[:, i, :, :], tf[:, i, :, :])
            # z = 1 - t (1 at zeros)
            z = bpool.tile([P, CB, W], BF16, tag=f"z{i}")
            nc.vector.tensor_scalar(
                z[:], tb[:, i, :, :], -1.0, 1.0, op0=AOP.mult, op1=AOP.add
            )
            # pass 1: S_c = B0' z_c + Bup' z_{c-1} + Bdn' z_{c+1} per block
            s_sb = bpool.tile([P, CB, W], BF16, tag=f"ssb{i}")
            for c in range(CB):
                ps_s = psum.tile([P, W], F32, tag="ps_s")
                terms = [(0, c)]
                if c > 0:
                    terms.append((1, c - 1))
                if c < CB - 1:
                    terms.append((2, c + 1))
                for j, (bidx, cz) in enumerate(terms):
                    nc.tensor.matmul(
                        ps_s[:], bsb[:, bidx, :], z[:, cz, :],
                        start=(j == 0), stop=(j == len(terms) - 1),
                    )
                nc.scalar.copy(s_sb[:, c, :], ps_s[:])
            # acc = sum_r [S >= 0.5 * 8^-r], r = 0..4  (= 5 - min(dist,5))
            acc = bpool.tile([P, CB, W], BF16, tag=f"acc{i}")
            nc.vector.tensor_single_scalar(acc[:], s_sb[:], 0.5, op=AOP.is_ge)
            for r in range(1, 5):
                ind = stage.tile([P, CB, W], BF16, tag="ind")
                nc.vector.tensor_single_scalar(
                    ind[:], s_sb[:], 0.5 * 8.0 ** (-r), op=AOP.is_ge
                )
                nc.vector.tensor_add(acc[:], acc[:], ind[:])
            # g = (5 - acc)^2
            g = bpool.tile([P, CB, W], BF16, tag=f"g{i}")
            nc.scalar.activation(
                g[:], acc[:], ACT.Square, bias=fives_f[:], scale=-1.0
            )
            # gs_phys[w+1] = g[w]: parity helper for aligned odd-k staging
            gs = bpool.tile([P, CB, W + 2], BF16, tag=f"gs{i}")
            nc.scalar.copy(gs[:, :, 1 : W + 1], g[:])
            nc.gpsimd.memset(gs[:, :, W + 1 :], BIGD)

            # pass 2: two accumulator chains (A: k in {0,1,4,5}, B: {2,3})
            def stage_k(k):
                kk = float(k * k)
                t2 = stage.tile([P, CB, W + 2 * k], BF16, tag="p2stage")
                nc.gpsimd.memset(t2[:, :, 0:k], BIGD)
                if k % 2 == 0:
                    nc.gpsimd.memset(t2[:, :, W + k :], BIGD)
                    nc.gpsimd.tensor_scalar_add(t2[:, :, k : W + k], g[:], kk)
                else:
                    # bulk from gs: both APs 4B-aligned, even count (4x)
                    if k > 1:
                        nc.gpsimd.memset(t2[:, :, W + k + 1 :], BIGD)
                    nc.vector.tensor_scalar_add(
                        t2[:, :, k + 1 : W + k + 1], gs[:, :, 2 : W + 2], kk
                    )
                    nc.vector.tensor_scalar_add(t2[:, :, k : k + 1], g[:, :, 0:1], kk)
                return t2

            A = bpool.tile([P, CB, W], BF16, tag=f"A{i}")
            Bt = bpool.tile([P, CB, W], BF16, tag=f"B{i}")
            for k in (1, 4, 5):
                t2 = stage_k(k)
                hi, lo = t2[:, :, 2 * k : 2 * k + W], t2[:, :, 0:W]
                nc.vector.tensor_tensor(
                    A[:], g[:] if k == 1 else A[:], hi, op=AOP.min
                )
                nc.vector.tensor_tensor(A[:], A[:], lo, op=AOP.min)
            for k in (2, 3):
                t2 = stage_k(k)
                hi, lo = t2[:, :, 2 * k : 2 * k + W], t2[:, :, 0:W]
                if k == 2:
                    nc.vector.tensor_tensor(Bt[:], hi, lo, op=AOP.min)
                else:
                    nc.vector.tensor_tensor(Bt[:], Bt[:], hi, op=AOP.min)
                    nc.vector.tensor_tensor(Bt[:], Bt[:], lo, op=AOP.min)
            nc.vector.tensor_tensor(A[:], A[:], Bt[:], op=AOP.min)
            Ds.append(A)

        if dbg_edt:
            Df = io.tile(SH4, F32, tag="Df")
            edt_dst = edt_dram.ap().rearrange("i (c p) w -> p i c w", p=P)
            for i in range(IMGS):
                nc.scalar.copy(Df[:, i, :, :], Ds[i][:])
                nc.sync.dma_start(edt_dst[:, i, :, :], Df[:, i, :, :])

        # ---- loss element maps (bf16) ----
        rowsums = small.tile([P, 2], F32, tag="rowsums")

        sub = bpool.tile(SH4, BF16, tag="sub")
        nc.vector.tensor_sub(sub[:], xb[:], tb[:])
        e = bpool.tile(SH4, BF16, tag="e")
        nc.scalar.activation(e[:], sub[:], ACT.Square, accum_out=rowsums[:, 0:1])
        pp = bpool.tile(SH4, BF16, tag="pp")
        nc.scalar.activation(pp[:], xb[:], ACT.Sigmoid, accum_out=rowsums[:, 1:2])
        y = bpool.tile(SH4, BF16, tag="y")
        nc.vector.tensor_mul(y[:], tb[:], e[:])
        scr = bpool.tile(SH4, BF16, tag="scr")
        nc.vector.tensor_mul(scr[:], pp[:], tb[:])
        # sum(p*t) and sum(t) via PE column sums (accumulated over images)
        ps_pt = psum.tile([1, W], F32, tag="pspt")
        ps_t = psum.tile([1, W], F32, tag="pst")
        n = 0
        for i in range(IMGS):
            for c in range(CB):
                first, last = n == 0, n == IMGS * CB - 1
                nc.tensor.matmul(
                    ps_pt[:], ones_b[:, 0:1], scr[:, i, c, :], start=first, stop=last
                )
                nc.tensor.matmul(
                    ps_t[:], ones_b[:, 0:1], tb[:, i, c, :], start=first, stop=last
                )
                n += 1

        # ---- per-image reductions ----
        # All scalar results land in one [1, 16] tile; one DMA at the end.
        # Layout: [vm0, vm1, dot0, dot1, s2_0, s2_1, S1, P, PT, T, ...]
        res_sb = small.tile([1, OUT_N], F32, tag="res_sb")

        # vmax over image of D: full reduce on GPSIMD (off the DVE)
        for i in range(IMGS):
            nc.gpsimd.tensor_reduce(
                res_sb[0:1, i : i + 1], Ds[i][:], axis=AXL.XYZWC, op=AOP.max
            )

        # per-image: selected row (row b_i < 16 lives in block c=0),
        # column sums of t*e, then dot and sum
        for i in range(IMGS):
            ps_drow = psum.tile([1, W], F32, tag=f"psdrow{i}")
            nc.tensor.matmul(
                ps_drow[:], selb[:, i : i + 1], Ds[i][:, 0, :],
                start=True, stop=True,
            )
            srow = small.tile([1, W], F32, tag=f"srow{i}")
            nc.scalar.sqrt(srow[:], ps_drow[:])

            ps_c = psum.tile([1, W], F32, tag=f"psc{i}")
            for c in range(CB):
                nc.tensor.matmul(
                    ps_c[:], ones_b[:, 0:1], y[:, i, c, :],
                    start=(c == 0), stop=(c == CB - 1),
                )

            scr2 = small.tile([1, W], F32, tag=f"scr2{i}")
            nc.vector.tensor_mul(scr2[:], srow[:], ps_c[:])
            nc.vector.reduce_sum(res_sb[0:1, 2 + i : 3 + i], scr2[:], axis=AXL.X)
            # s2 via ACT accumulate (keeps the DVE free)
            scr3 = small.tile([1, W], F32, tag=f"scr3{i}")
            nc.scalar.activation(
                scr3[:], ps_c[:], ACT.Identity,
                accum_out=res_sb[0:1, 4 + i : 5 + i],
            )

        # global sums: [S1, sumP] from ACT row accums via PE; [PT, T] via ACT
        ps_sums = psum.tile([1, 2], F32, tag="pssums")
        nc.tensor.matmul(ps_sums[:], ones_f[:, 0:1], rowsums[:], start=True, stop=True)
        nc.scalar.copy(res_sb[0:1, 6:8], ps_sums[:])
        scr4 = small.tile([1, W], F32, tag="scr4")
        nc.scalar.activation(
            scr4[:], ps_pt[:], ACT.Identity, accum_out=res_sb[0:1, 8:9]
        )
        scr5 = small.tile([1, W], F32, tag="scr5")
        nc.scalar.activation(
            scr5[:], ps_t[:], ACT.Identity, accum_out=res_sb[0:1, 9:10]
        )

        # ---- write results ----
        nc.sync.dma_start(res_dram.ap()[0:1, 0:10], res_sb[0:1, 0:10])

    nc.compile()
    return nc


_NC_CACHE = {}


def _get_nc():
    key = os.environ.get("KERNEL_DEBUG_EDT") == "1"
    if key not in _NC_CACHE:
        _NC_CACHE[key] = _build_nc()
    return _NC_CACHE[key]


def _make_sel(core_id):
    sel = np.zeros((P, IMGS), dtype=np.float32)
    for i in range(IMGS):
        b = IMGS * core_id + i
        sel[b, i] = 1.0  # row b is (c=0, p=b) since b < 16
    return sel


def kernel(inputs, targets):
    nc = _get_nc()
    in_maps = []
    for core in range(NCORES):
        sl = slice(IMGS * core, IMGS * (core + 1))
        in_maps.append(
            {
                "x": np.ascontiguousarray(inputs[sl, 0]).astype(np.float32),
                "t": np.ascontiguousarray(targets[sl, 0]).astype(np.float32),
                "sel": _make_sel(core),
            }
        )

    trace = os.environ.get("KERNEL_TRACE") == "1"
    if trace:
        try:  # NTFF tracing needs the axon hook; absent in some containers
            from antenv.axon_hooks import get_axon_ntff_profile_hook  # noqa: F401
        except ImportError:
            trace = False
    run_res = run_bass_kernel_spmd(
        nc, in_maps, core_ids=list(range(NCORES)), trace=trace
    )
    results = run_res.results
    if trace and run_res.exec_time_ns is not None:
        print(f"HW exec time: {run_res.exec_time_ns} ns")
        kernel.last_exec_time_ns = run_res.exec_time_ns

    wnum = 0.0
    s1 = sp = spt = st = 0.0
    for core in range(NCORES):
        r = np.asarray(results[core]["res"], dtype=np.float64)[0]
        for i in range(IMGS):
            v = np.sqrt(r[i])
            wnum += v * r[4 + i] - r[2 + i]
        s1 += r[6]
        sp += r[7]
        spt += r[8]
        st += r[9]

    wmse = (wnum + EPS_W * s1) / float(B * C * H * W)
    dice = 1.0 - (2.0 * spt + SMOOTH) / (sp + st + SMOOTH)

    if os.environ.get("KERNEL_DEBUG_EDT") == "1":
        kernel.last_edt2 = np.concatenate(
            [np.asarray(results[c]["edt2"]) for c in range(NCORES)], axis=0
        )

    return (np.float32(ALPHA * wmse), np.float32(BETA * dice))



# revision 6
# speedup vs baseline: 2.2929x; 2.2929x over previous
"""Trainium2 Bass kernel for the distance-transform loss.

For inputs/targets (16, 1, 512, 512):
    wmse = ALPHA * mean(weight * (inputs - targets)^2)
    dice = BETA  * (1 - (2*I + S) / (U + S))
weight is built from the per-sample EDT of targets (distance to nearest
zero): v_b = max(edt_b), row_b = edt_b[row=b, :], mask = (edt != 0) = t,
weight = mask * (v_b - row_b[w]) + EPS_W.

Algorithm (validated against the reference in fp64):
  * The EDT^2 map D is only needed through max(D) and one row per image.
    Both are recovered from V = z (*) K, the 2D convolution of
    z = (t == 0) with the separable kernel K[j,k] = 16^{-(j^2+k^2)},
    |j|,|k| <= 3:  D = ceil(-log16(V) - eps) exactly wherever the true
    D <= 10 (lattice-point multiplicity at each squared radius is < 16,
    so the leading base-16 term dominates the sum).  The data here has
    max D = 10; if the decode exceeds 10 anywhere the host falls back
    to an exact numpy recomputation.
  * Pass 1 (vertical) runs on the PE with the image data as the
    STATIONARY operand and the banded weights as MOVING, so its output
    is already transposed (w on partitions) and pass 2 (horizontal) is
    a second banded matmul — no separate transpose pass.
  * max(D) = decode(min V): per-chunk min-reduces of V (DVE), decoded
    on the host; the 16 candidate rows b < 16 are one PSUM column
    slice, decoded host-side in float64.
  * Loss sums:  S1 = sum((x-t)^2) = sum(q) + 2*sum(u) with
    q = (x-1)^2, u = (x-0.5)*z (DVE fused multiply + accumulate);
    t*e = t*q;  per-column C = colsum(t*q) on the PE feeds
    dot(sqrt(Drow), C) on the host;  sum(p*t) via colsum(p*t).

Sharding: data-parallel, 2 images per core on 8 cores; per-core scalar
partials are combined on the host (the all-reduce-mean step).
"""

import os
from contextlib import ExitStack

import ml_dtypes
import numpy as np

import concourse.bacc as bacc
import concourse.bass as bass
import concourse.mybir as mybir
import concourse.tile as tile
from concourse.bass_utils import run_bass_kernel_spmd

# Problem constants (hardcoded per the task contract).
B, C, H, W = 16, 1, 512, 512
NCORES = 8
IMGS = B // NCORES          # images per core
CB = 4                      # 512 rows = 4 blocks of 128: h = 128*c + p
P = 128
R = 3                       # conv radius; exact while max EDT^2 <= 10
EPS_W = 1e-3
SMOOTH = 1e-6
ALPHA = 0.6
BETA = 1.0
LOG16 = np.log(16.0)

F32 = mybir.dt.float32
BF16 = mybir.dt.bfloat16
AOP = mybir.AluOpType
ACT = mybir.ActivationFunctionType
AXL = mybir.AxisListType

# staging [128, 16] f32 columns:
#   0,1  z-op accum per img: acc_p = sum_free(-tb) + 1  ->  sumT_i = sum_p(1-acc)
#   2,3  sumP rows | 4,5 sum_q rows | 6,7 sum_u rows
#   8,9  minV img0 halves | 10,11 minV img1 halves  (column min over partitions)
ST_N = 16


def _band_weights():
    """[128, 3, 128] bf16 band matrices for both conv passes.
    [:, 0, :]: within-block  w(m - p)
    [:, 1, :]: in-block above out-block   w(128 + m - p)
    [:, 2, :]: in-block below out-block   w(-128 + m - p)
    with w(d) = 16^-(d*d) for |d| <= R else 0."""
    def w(d):
        return 16.0 ** -(d * d) if abs(d) <= R else 0.0

    bm = np.zeros((P, 3, P), np.float32)
    for p in range(P):
        for m in range(max(0, p - R), min(P, p + R + 1)):
            bm[p, 0, m] = w(m - p)
    for m in range(0, R):
        for p in range(P - R + m, P):
            bm[p, 1, m] = w(128 + m - p)
    for p in range(0, R):
        for m in range(P - R + p, P):
            bm[p, 2, m] = w(m - 128 - p)
    return bm.astype(ml_dtypes.bfloat16)


def _build_nc():
    nc = bacc.Bacc(
        "TRN2",
        target_bir_lowering=False,
        debug=False,
        num_devices=NCORES,
    )

    x_dram = nc.dram_tensor("x", [IMGS, H, W], F32, kind="ExternalInput")
    t_dram = nc.dram_tensor("t", [IMGS, H, W], F32, kind="ExternalInput")
    res1_dram = nc.dram_tensor("res1", [P, ST_N], F32, kind="ExternalOutput")
    res2_dram = nc.dram_tensor("res2", [P, IMGS * CB * 16], F32, kind="ExternalOutput")
    res3_dram = nc.dram_tensor("res3", [1, 3 * W], F32, kind="ExternalOutput")

    with tile.TileContext(nc) as tc, ExitStack() as ctx:
        io = ctx.enter_context(tc.tile_pool(name="io", bufs=1))
        bpool = ctx.enter_context(tc.tile_pool(name="b16", bufs=1))
        small = ctx.enter_context(tc.tile_pool(name="small", bufs=1))
        vvt_ps = ctx.enter_context(
            tc.tile_pool(name="vvtps", bufs=1, space=bass.MemorySpace.PSUM)
        )
        v_ps = ctx.enter_context(
            tc.tile_pool(name="vps", bufs=1, space=bass.MemorySpace.PSUM)
        )
        row_ps = ctx.enter_context(
            tc.tile_pool(name="rowps", bufs=1, space=bass.MemorySpace.PSUM)
        )

        SH4 = [P, IMGS, CB, W]   # (p, i, c, w), h = 128c + p

        # ---- loads: t first (EDT critical path), then x per c-block ----
        xf = io.tile(SH4, F32, tag="xf")
        tf = io.tile(SH4, F32, tag="tf")
        x_src = x_dram.ap().rearrange("i (c p) w -> p i c w", p=P)
        t_src = t_dram.ap().rearrange("i (c p) w -> p i c w", p=P)
        for i in range(IMGS):
            nc.sync.dma_start(tf[:, i, :, :], t_src[:, i, :, :])
        for i in range(IMGS):
            for c in range(CB):
                nc.sync.dma_start(xf[:, i, c, :], x_src[:, i, c, :])

        bmat_dram = nc.inline_tensor(_band_weights(), name="bweights")
        bsb = small.tile([P, 3, P], BF16, tag="bsb")
        nc.sync.dma_start(bsb[:], bmat_dram.ap())

        ones_b = small.tile([P, 1], BF16, tag="onesb")
        nc.gpsimd.memset(ones_b[:], 1.0)
        negones_f = small.tile([P, 1], F32, tag="negones")
        nc.gpsimd.memset(negones_f[:], -1.0)

        staging = small.tile([P, ST_N], F32, tag="staging")
        stag2 = small.tile([P, IMGS, CB, 16], F32, tag="stag2")
        crow = small.tile([1, 3 * W], F32, tag="crow")

        tb = bpool.tile(SH4, BF16, tag="tb")
        z = bpool.tile(SH4, BF16, tag="z")
        q = bpool.tile(SH4, BF16, tag="q")
        pp = bpool.tile(SH4, BF16, tag="pp")
        y = bpool.tile(SH4, BF16, tag="y")
        scr = bpool.tile(SH4, BF16, tag="scr")
        junk_u = bpool.tile(SH4, BF16, tag="junku")
        vvt_sb = bpool.tile([P, IMGS, CB, W], BF16, tag="vvtsb")  # [w, i, wb, h']

        # crow psum strips: cy (both images) + cs
        cy_ps = row_ps.tile([1, IMGS, W], F32, tag="cyps")
        cs_ps = row_ps.tile([1, W], F32, tag="csps")

        def band_idx(delta):
            # in-block minus out-block: -1 -> above (1), +1 -> below (2)
            return {0: 0, -1: 1, 1: 2}[delta]

        # ---- per-image pipelines ----
        for i in range(IMGS):
            # tb = bf16(t) on Pool; z = 1 - tb on DVE (accum quirk -> sumT)
            nc.gpsimd.tensor_copy(tb[:, i, :, :], tf[:, i, :, :])
            nc.vector.tensor_scalar(
                z[:, i, :, :], tb[:, i, :, :], -1.0, 1.0,
                op0=AOP.mult, op1=AOP.add,
                accum_out=staging[:, 0 + i : 1 + i],
            )

            for half in range(2):   # h'-chunks of 256
                # pass 1: VvT[w, wb, h'] = sum_h z[h, w] * K(h' - h)
                vvt_c = vvt_ps.tile([P, CB, 2 * P], F32, tag="vvt")
                for sub in range(2):
                    hp = 2 * half + sub
                    for wb in range(CB):
                        srcs = [hb for hb in (hp - 1, hp, hp + 1) if 0 <= hb < CB]
                        for n, hb in enumerate(srcs):
                            nc.tensor.matmul(
                                vvt_c[:, wb, sub * P : (sub + 1) * P],
                                z[:, i, hb, wb * P : (wb + 1) * P],
                                bsb[:, band_idx(hb - hp), :],
                                start=(n == 0), stop=(n == len(srcs) - 1),
                            )
                # copy half-chunk to SBUF bf16 (Act; frees the PSUM banks)
                nc.scalar.activation(
                    vvt_sb[:, i, :, half * 2 * P : (half + 1) * 2 * P],
                    vvt_c[:], ACT.Copy,
                )

            for half in range(2):   # h-chunks of the final V
                # pass 2: V[w', wpb, h] = sum_w K(w' - w) * VvT[w, wb, h]
                v_c = v_ps.tile([P, CB, 2 * P], F32, tag="vchunk")
                for sub in range(2):
                    hc = 2 * half + sub
                    for wpb in range(CB):
                        srcs = [wb for wb in (wpb - 1, wpb, wpb + 1) if 0 <= wb < CB]
                        for n, wb in enumerate(srcs):
                            nc.tensor.matmul(
                                v_c[:, wpb, sub * P : (sub + 1) * P],
                                bsb[:, band_idx(wb - wpb), :],
                                vvt_sb[:, i, wb, hc * P : (hc + 1) * P],
                                start=(n == 0), stop=(n == len(srcs) - 1),
                            )
                if half == 0:
                    # candidate rows b < 16 for the weight-row selection
                    nc.vector.tensor_scalar_add(
                        stag2[:, i, :, :], v_c[:, :, 0:16], 0.0
                    )
                # min V over the half -> host decodes max EDT^2
                nc.vector.tensor_reduce(
                    staging[:, 8 + 2 * i + half : 9 + 2 * i + half],
                    v_c[:], axis=AXL.XYZW, op=AOP.min,
                )

        # ---- loss maps (x-dependent; split for DMA-tail pipelining) ----
        for i in range(IMGS):
            for hf in range(2):
                cs_ = slice(2 * hf, 2 * hf + 2)
                nc.scalar.activation(
                    pp[:, i, cs_, :], xf[:, i, cs_, :], ACT.Sigmoid,
                    accum_out=staging[:, 2 + i : 3 + i],
                )
                nc.scalar.activation(
                    q[:, i, cs_, :], xf[:, i, cs_, :], ACT.Square,
                    bias=negones_f[:], scale=1.0,
                    accum_out=staging[:, 4 + i : 5 + i],
                )
                # u = (x - 0.5) * z  (S1 = sum q + 2 sum u)
                nc.vector.scalar_tensor_tensor(
                    junk_u[:, i, cs_, :], xf[:, i, cs_, :], 0.5, z[:, i, cs_, :],
                    op0=AOP.subtract, op1=AOP.mult,
                    accum_out=staging[:, 6 + i : 7 + i],
                )
                if i == 0:
                    nc.gpsimd.tensor_tensor(
                        y[:, i, cs_, :], tb[:, i, cs_, :], q[:, i, cs_, :],
                        op=AOP.mult,
                    )
                else:
                    nc.vector.tensor_mul(
                        y[:, i, cs_, :], tb[:, i, cs_, :], q[:, i, cs_, :]
                    )
                nc.vector.tensor_mul(
                    scr[:, i, cs_, :], pp[:, i, cs_, :], tb[:, i, cs_, :]
                )

        # column sums: C_i = colsum(y_i), Cs = colsum(p*t) over both images
        for i in range(IMGS):
            for c in range(CB):
                nc.tensor.matmul(
                    cy_ps[:, i, :], ones_b[:, 0:1], y[:, i, c, :],
                    start=(c == 0), stop=(c == CB - 1),
                )
        n = 0
        for i in range(IMGS):
            for c in range(CB):
                nc.tensor.matmul(
                    cs_ps[:], ones_b[:, 0:1], scr[:, i, c, :],
                    start=(n == 0), stop=(n == IMGS * CB - 1),
                )
                n += 1
        nc.scalar.activation(crow[0:1, 0 : 2 * W], cy_ps[0:1, :, :], ACT.Copy)
        nc.scalar.activation(crow[0:1, 2 * W : 3 * W], cs_ps[:], ACT.Copy)

        # ---- write results ----
        nc.sync.dma_start(
            res2_dram.ap().rearrange("p (i c s) -> p i c s", i=IMGS, c=CB),
            stag2[:],
        )
        nc.sync.dma_start(res1_dram.ap(), staging[:])
        nc.sync.dma_start(res3_dram.ap(), crow[:])

    nc.compile()
    return nc


_NC_CACHE = {}


def _get_nc():
    if "nc" not in _NC_CACHE:
        _NC_CACHE["nc"] = _build_nc()
    return _NC_CACHE["nc"]


def _decode(v):
    """ceil(-log16(V) - eps): exact EDT^2 for values <= 10 (eps absorbs
    bf16 rounding; band multiplicity keeps -log16 V within (D-1, D])."""
    v = np.maximum(np.asarray(v, np.float64), 1e-300)
    return np.ceil(-np.log(v) / LOG16 - 0.01)


def _fallback(inputs, targets):
    """Exact numpy recomputation (triggers only if some EDT^2 > 10)."""
    x = inputs[:, 0].astype(np.float64)
    t = targets[:, 0].astype(np.float64)
    Bn, Hn, Wn = t.shape
    z = t == 0
    big = 1e8
    d = np.full((Bn, Hn, Wn), big)
    run = np.full((Bn, Wn), big)
    for h in range(Hn):
        run = np.where(z[:, h, :], 0.0, run + 1.0)
        d[:, h, :] = run
    run = np.full((Bn, Wn), big)
    for h in range(Hn - 1, -1, -1):
        run = np.where(z[:, h, :], 0.0, run + 1.0)
        d[:, h, :] = np.minimum(d[:, h, :], run)
    g = np.minimum(d * d, big)
    ys = np.arange(Wn, dtype=np.float64)
    out = np.full((Bn, Hn, Wn), big)
    for j in range(Wn):
        cand = g[:, :, j][:, :, None] + (ys - j)[None, None, :] ** 2
        out = np.minimum(out, cand)
    w_edt = np.sqrt(out)
    v = w_edt.max(axis=(1, 2))
    rows = w_edt[np.arange(Bn), np.arange(Bn), :]
    mask = (w_edt != 0).astype(np.float64)
    wgt = mask * (v[:, None, None] - rows[:, None, :]) + EPS_W
    wmse = float((wgt * (x - t) ** 2).mean())
    p = 1.0 / (1.0 + np.exp(-x))
    inter = float((p * t).sum())
    union = float(p.sum() + t.sum())
    dice = 1.0 - (2.0 * inter + SMOOTH) / (union + SMOOTH)
    return np.float32(ALPHA * wmse), np.float32(BETA * dice)


def kernel(inputs, targets):
    nc = _get_nc()
    in_maps = []
    for core in range(NCORES):
        sl = slice(IMGS * core, IMGS * (core + 1))
        in_maps.append(
            {
                "x": np.ascontiguousarray(inputs[sl, 0]).astype(np.float32),
                "t": np.ascontiguousarray(targets[sl, 0]).astype(np.float32),
            }
        )

    trace = os.environ.get("KERNEL_TRACE") == "1"
    if trace:
        try:  # NTFF tracing needs the axon hook; absent in some containers
            from antenv.axon_hooks import get_axon_ntff_profile_hook  # noqa: F401
        except ImportError:
            trace = False
    run_res = run_bass_kernel_spmd(
        nc, in_maps, core_ids=list(range(NCORES)), trace=trace
    )
    results = run_res.results
    if trace and run_res.exec_time_ns is not None:
        print(f"HW exec time: {run_res.exec_time_ns} ns")
        kernel.last_exec_time_ns = run_res.exec_time_ns

    wnum = 0.0
    sT = sP = sQ = sU = sPT = 0.0
    ok = True
    for core in range(NCORES):
        r1 = np.asarray(results[core]["res1"], np.float64)
        r2 = np.asarray(results[core]["res2"], np.float64).reshape(P, IMGS, CB, 16)
        r3 = np.asarray(results[core]["res3"], np.float64)[0]
        # z-op accum quirk: acc_p = sum_free(-tb_p) + 1  ->  rowT = 1 - acc
        sT += (1.0 - r1[:, 0:2]).sum()
        sP += r1[:, 2:4].sum()
        sQ += r1[:, 4:6].sum()
        sU += r1[:, 6:8].sum()
        sPT += r3[2 * W : 3 * W].sum()
        for i in range(IMGS):
            b = IMGS * core + i
            minv = r1[:, 8 + 2 * i : 10 + 2 * i].min()
            vmax2 = _decode(minv)
            if not (minv > 0 and vmax2 <= 10):
                ok = False
                continue
            vb = r2[:, i, :, b].T.reshape(W)       # V at row b, w = 128*wpb + p
            drow = _decode(vb)
            ci = r3[i * W : (i + 1) * W]
            wnum += np.sqrt(vmax2) * ci.sum() - (np.sqrt(drow) * ci).sum()

    if not ok:
        return _fallback(np.asarray(inputs), np.asarray(targets))

    s1 = sQ + 2.0 * sU
    wmse = (wnum + EPS_W * s1) / float(B * C * H * W)
    dice = 1.0 - (2.0 * sPT + SMOOTH) / (sP + sT + SMOOTH)
    return (np.float32(ALPHA * wmse), np.float32(BETA * dice))
